# revision 8
# baseline (speedup 1.0000x reference)
"""Trainium2 Bass kernel for FastUserEmbedding attention pooling (V6).

Problem: B=4096, L=200, D=128 fp32, lengths-masked softmax attention pooling
followed by LayerNorm.  Data-parallel over 8 NeuronCores (512 rows each).

V6 design goal: MINIMAL INSTRUCTION COUNT (~230 vs 6645 in V4) while staying
near the fp16 HBM roofline.  The V4 baseline spent thousands of tiny PE
matmuls (plus one Ldweights each); on real HW the graded time tracked
instruction count, not the cost model.  V6 uses no PE at all:

  scores  xw = x * w  (tensor_tensor, w broadcast via stride-0 outer AP)
          d-add-tree over xw in place (7 tensor_tensor adds)
  exp     ACT, strided read of the tree root at xw[:, :, 0]
  exm2    exp * mask, pair-duplicated fp16 (one DVE op; the duplication
          keeps the premul broadcast AP innermost step 1 => 2x perf mode)
  premul  x *= exm2  in place (DVE tensor_tensor, 4D APs)
  pool    l-add-tree over x in place (8 tensor_tensor adds)
  LN      on the fp16 pooled vector (softmax denominator cancels in LN)

The add-trees run on GPSIMD ('standard' library tensor_tensor) or DVE per a
static split chosen to balance the two engines.

Math restructure as in V4: softmax is shift-invariant and LayerNorm is
scale-invariant, so LN(sum_l softmax(s)_l x_l) = LN(sum_l exp(s_l) m_l x_l).

walrus allows ONE semaphore wait per instruction; fix_waits applies
per-instruction keep policies recorded at build time (each pruned wait is
transitively implied -- see inline notes).
"""

import numpy as np

B, L, D = 4096, 200, 128
N_CORES = 8
B_SHARD = B // N_CORES          # 512
N_BLK = B_SHARD // 128          # 4
LH = L // 2                     # 100, half-block l-chunk
LN_EPS = 1e-5

# fp16 const blob layout (offsets in elements)
O_W = 0              # [128, 128]  w replicated per partition
O_GB = 128           # [128, 128]  gamma
O_BB = 256           # [128, 128]  beta
F16TOT = 384
# fp32 const blob layout
O_IOTA = 0           # [128, 200]
O_LEN = 200          # [128, 4]
F32TOT = 204


def build_v6(dtree_gp=frozenset(), ltree_gp=frozenset(), x_bufs=3,
             debug_taps=False):
    """dtree_gp: steps (block-halves) whose d-tree runs on GPSIMD.
    ltree_gp: blocks whose l-tree runs on GPSIMD.
    debug_taps: add DRAM dumps of ex/exm2/pc per block."""
    import concourse.bass as bass
    import concourse.tile as tile
    import concourse.mybir as mybir

    f32 = mybir.dt.float32
    f16 = mybir.dt.float16
    Alu = mybir.AluOpType
    Act = mybir.ActivationFunctionType
    X = mybir.AxisListType.X

    use_pool = bool(dtree_gp) or bool(ltree_gp)

    nc = bass.Bass("TRN2", target_bir_lowering=False, debug=False)

    x_d = nc.dram_tensor("x", [B_SHARD, L, D], f16, kind="ExternalInput")
    cb16_d = nc.dram_tensor("cb16", [128, F16TOT], f16, kind="ExternalInput")
    cb32_d = nc.dram_tensor("cb32", [128, F32TOT], f32, kind="ExternalInput")
    out_d = nc.dram_tensor("out", [B_SHARD, D], f32, kind="ExternalOutput")

    x_ap = x_d.ap()
    out_ap = out_d.ap()
    dbg = {}
    if debug_taps:
        dbg["ex"] = nc.dram_tensor("dbg_ex", [N_BLK, 128, L], f16,
                                   kind="ExternalOutput")
        dbg["exm2"] = nc.dram_tensor("dbg_exm2", [N_BLK, 128, L, 2], f16,
                                     kind="ExternalOutput")
        dbg["pc"] = nc.dram_tensor("dbg_pc", [N_BLK, 128, D], f16,
                                   kind="ExternalOutput")
        dbg["oall"] = nc.dram_tensor("dbg_oall", [128, N_BLK * D], f32,
                                     kind="ExternalOutput")
        dbg["rstd"] = nc.dram_tensor("dbg_rstd", [N_BLK, 128, 1], f32,
                                     kind="ExternalOutput")
        dbg["mean"] = nc.dram_tensor("dbg_mean", [N_BLK, 128, 1], f32,
                                     kind="ExternalOutput")

    # wait policy: inst -> keep rule ("ACT"/"DVE"/"Pool"/"DMAHW"/"none")
    policy = []

    def pol(inst, keep):
        policy.append((inst, keep))
        return inst

    with tile.TileContext(nc) as tc:
        with (
            tc.tile_pool(name="const", bufs=1) as constp,
            tc.tile_pool(name="x", bufs=x_bufs) as xp,
            tc.tile_pool(name="xw", bufs=1) as xwp,
            tc.tile_pool(name="blk", bufs=2) as blkp,
            tc.tile_pool(name="small", bufs=2) as sp,
            tc.tile_pool(name="one", bufs=1) as onep,
        ):
            cb16_t = constp.tile([128, F16TOT], f16, tag="cb16")
            nc.sync.dma_start(cb16_t[:], cb16_d.ap())
            cb32_t = constp.tile([128, F32TOT], f32, tag="cb32")
            nc.sync.dma_start(cb32_t[:], cb32_d.ap())

            w_t = cb16_t[:, O_W:O_W + D]
            gb_t = cb16_t[:, O_GB:O_GB + D]
            bb_t = cb16_t[:, O_BB:O_BB + D]
            iota_t = cb32_t[:, O_IOTA:O_IOTA + L]
            len_t = cb32_t[:, O_LEN:O_LEN + N_BLK]

            if use_pool:
                from concourse import library_config
                nc.gpsimd.load_library(library_config.standard)

            # DVE probe absorbs the cb16 DMA wait (w/gb/bb reads)
            pjv = onep.tile([128, 1], f16, tag="pjv")
            nc.vector.tensor_copy(pjv[:], cb16_t[:, 0:1])

            # masks: mask[k][b, l] = (l < len[b, k]) as f32; the first carries
            # the cb32 DMA wait, the rest are covered by DVE order
            mask_t = {}
            for k in range(N_BLK):
                m = onep.tile([128, L], f32, tag=f"mask{k}")
                nc.vector.tensor_scalar(
                    out=m[:], in0=iota_t, scalar1=len_t[:, k:k + 1],
                    scalar2=None, op0=Alu.is_lt)
                mask_t[k] = m

            eps_t = onep.tile([128, 1], f32, tag="eps")
            nc.vector.memset(eps_t[:], LN_EPS)
            o_all = onep.tile([128, N_BLK * D], f32, tag="o_all")

            xt = {}          # block -> x tile
            xwt = {}         # step -> xw tile
            sct = {}         # block -> scores tile
            ext = {}         # block -> ex tile

            def dma_block(k):
                t = xp.tile([128, L, D], f16, tag="x", name=f"x{k}")
                # re-DMA into a recycled slot: the evicted block's last
                # reader is its pooled reduce (DVE) issued just before this
                d = nc.sync.dma_start(
                    t[:], x_ap[k * 128:(k + 1) * 128, :, :])
                if k >= x_bufs:
                    pol(d, "DVE")
                xt[k] = t

            def step_scores(s):
                k, h = divmod(s, 2)
                x_t = xt[k]
                xs = x_t[:, h * LH:(h + 1) * LH, :]
                if h == 0:
                    # DVE probe absorbs the block's x DMA wait
                    nc.vector.tensor_copy(pjv[:], xs[:, 0, 0:1])
                xw = xwp.tile([128, LH, D], f16, tag="xw", name=f"xw{s}")
                wb = w_t.unsqueeze(1).broadcast_to([128, LH, D])
                # the xw buffer's last reader is the d-reduce (DVE), so the
                # overwrite needs no cross-engine wait
                ti = nc.vector.tensor_tensor(out=xw[:], in0=xs, in1=wb,
                                             op=Alu.mult)
                pol(ti, "none")
                xwt[s] = xw
                # d add-tree in place down to width 16, then one tensor_reduce
                # into a compact f32 scores tile
                wd = D // 2
                while wd >= 16:
                    nc.vector.tensor_tensor(
                        out=xw[:, :, 0:wd], in0=xw[:, :, 0:wd],
                        in1=xw[:, :, wd:2 * wd], op=Alu.add)
                    wd //= 2
                if h == 0:
                    sct[k] = blkp.tile([128, L], f32, tag="sc",
                                       name=f"sc{k}")
                sc = sct[k]
                ri = nc.vector.reduce_sum(
                    sc[:, h * LH:(h + 1) * LH], xw[:, :, 0:16], axis=X)
                # sc slot WAR: exp(k-2) (ACT) read it
                pol(ri, "ACT" if (h == 0 and k >= 2) else "none")
                if h == 1:
                    # one exp per block: contiguous f32 scores -> fp16 ex
                    ext[k] = blkp.tile([128, L], f16, tag="ex",
                                       name=f"ex{k}")
                    ei = nc.scalar.activation(ext[k][:], sc[:], Act.Exp)
                    # keep the reduce wait; the ex-slot WAR (exm2_tt(k-2)
                    # read it, DVE) is implied through the reduce chain
                    pol(ei, "DVE")

            def step_premul(k):
                import concourse.bass as bass_mod
                x_t = xt[k]
                ex = ext[k]
                # exm2 = (exp * mask) pair-duplicated to fp16 in one op
                exm2 = sp.tile([128, L, 2], f16, tag="exm2", name=f"exm2{k}")
                e_in = ex[:].unsqueeze(2).broadcast_to([128, L, 2])
                m_in = mask_t[k][:].unsqueeze(2).broadcast_to([128, L, 2])
                mi = nc.vector.tensor_tensor(out=exm2[:], in0=e_in, in1=m_in,
                                             op=Alu.mult)
                pol(mi, "ACT")  # exp(2k+1); exm2-slot WAR (premul k-2, DVE own)
                # DVE premul in place on x (4D APs keep innermost step 1)
                e4 = bass_mod.AP(exm2[:].tensor, exm2[:].offset,
                                 [[2 * L, 128], [2, L], [0, D // 2], [1, 2]])
                x4 = bass_mod.AP(x_t[:].tensor, x_t[:].offset,
                                 [[L * D, 128], [D, L], [2, D // 2], [1, 2]])
                ti = nc.vector.tensor_tensor(out=x4, in0=x4, in1=e4,
                                             op=Alu.mult)
                # WAR vs the slot re-DMA readers is own-DVE-order; the x DMA
                # wait came through this half's DVE probe
                pol(ti, "none")
                if debug_taps:
                    nc.sync.dma_start(dbg["ex"].ap()[k], ex[:])
                    nc.sync.dma_start(dbg["exm2"].ap()[k], exm2[:])
                return ti

            def step_ltree(k, premul_inst):
                x_t = xt[k]
                import concourse.bass as bass_mod
                for wd in (72, 64, 32):
                    nc.vector.tensor_tensor(
                        out=x_t[:, 0:wd, :], in0=x_t[:, 0:wd, :],
                        in1=x_t[:, (128 if wd == 72 else wd):
                                (200 if wd == 72 else 2 * wd), :],
                        op=Alu.add)
                # strided reduce over the remaining 32 l rows straight into
                # the f32 pooled tile (frees the x slot at reduce end; the
                # slot re-DMA waits only on this)
                pc = sp.tile([128, D], f32, tag="pc", name=f"pc{k}")
                xv = bass_mod.AP(x_t[:].tensor, x_t[:].offset,
                                 [[L * D, 128], [1, D], [D, 32]])
                ri = nc.vector.reduce_sum(pc[:], xv, axis=X)
                pol(ri, "none")
                if debug_taps:
                    nc.sync.dma_start(dbg["pc"].ap()[k], pc[:])
                return pc

            def ln_blk(k, pc):
                pooled = pc[:]                 # f32 [128, D]
                # mean/var in two DVE ops (bn_stats even/odd partial stats,
                # bn_aggr combines them); mv = [mean, var]
                stats = sp.tile([128, 6], f32, tag="bnst", name=f"bnst{k}")
                nc.vector.bn_stats(stats[:], pooled)
                mv = sp.tile([128, 2], f32, tag="bnmv", name=f"bnmv{k}")
                nc.vector.bn_aggr(mv[:], stats[:])
                mean = mv[:, 0:1]
                std = sp.tile([128, 1], f32, tag="std", name=f"std{k}")
                nc.scalar.activation(std[:], mv[:, 1:2], Act.Sqrt,
                                     bias=eps_t[:], scale=1.0)
                rstd = sp.tile([128, 1], f32, tag="rstd", name=f"rstd{k}")
                ci = nc.vector.reciprocal(rstd[:], std[:])
                pol(ci, "ACT")
                o1 = sp.tile([128, D], f32, tag="o1", name=f"o1_{k}")
                oi = nc.vector.scalar_tensor_tensor(
                    out=o1[:], in0=pooled, scalar=mean, in1=gb_t,
                    op0=Alu.subtract, op1=Alu.mult)
                pol(oi, "none")
                nc.vector.scalar_tensor_tensor(
                    out=o_all[:, k * D:(k + 1) * D],
                    in0=o1[:], scalar=rstd[:], in1=bb_t,
                    op0=Alu.mult, op1=Alu.add)
                if debug_taps:
                    nc.sync.dma_start(dbg["rstd"].ap()[k], rstd[:])
                    nc.sync.dma_start(dbg["mean"].ap()[k], mean[:])

            # ---------------- schedule ----------------
            # NOTE: re-DMAs into recycled x slots MUST be issued after the
            # evicted block's last reader exists (tile computes waits at
            # issue time; issuing the DMA mid-block-k-2 made it race the
            # premul/ltree of the evicted block).  The pooled copy-out (pc)
            # is that last reader, so re-DMAs go right after it.
            for k in range(min(x_bufs, N_BLK)):
                dma_block(k)
            for k in range(N_BLK):
                for h in (0, 1):
                    step_scores(2 * k + h)
                pi = step_premul(k)
                pc = step_ltree(k, pi)
                if k + x_bufs < N_BLK:
                    dma_block(k + x_bufs)
                ln_blk(k, pc)

            if debug_taps:
                nc.sync.dma_start(dbg["oall"].ap(), o_all[:])
            out_dma = nc.sync.dma_start(
                out_ap.rearrange("(blk p) d -> p blk d", p=128), o_all[:])
            pol(out_dma, "DVE")

    fix_waits(nc, out_dma, policy)
    if use_pool:
        from concourse.library_overlay import lower_extended_insts
        lower_extended_insts(nc)
    return nc, out_dma


def _eng(w):
    """Engine prefix of a wait's semaphore name: 'DMAHW3_44' -> 'DMAHW'."""
    return w.ant_name.split("_")[0].rstrip("0123456789")


def fix_waits(nc, out_dma, policy):
    """Prune semaphore waits to <=1 per instruction (walrus codegen limit).

    Generic rules: same-engine waits are redundant (engines execute in
    order); a DMACopy's wait on its own queue sem is redundant (FIFO per
    queue); the final Drain keeps only the out-DMA's queue sem (all engine
    chains flow into the final o_all STT the out-DMA waits on).  Explicit
    per-instruction policies (recorded at build) choose which cross-engine
    wait to keep; every dropped wait is transitively implied by the kept one
    (see build_v6 inline notes).
    """
    out_q = {w.ant_name for w in (out_dma.ins.sync_info.on_update or [])
             if w.ant_name.startswith("DMAHW")}
    assert len(out_q) == 1, f"out dma queue sems: {out_q}"
    pol_map = {}
    for inst, keep in policy:
        pol_map[inst.ins.name] = keep
    eng_map = {"Activation": "Activation", "DVE": "DVE", "Pool": "Pool",
               "PE": "PE", "SP": "SP"}
    keep_map = {"ACT": "Activation", "DVE": "DVE", "Pool": "Pool",
                "PE": "PE", "DMAHW": "DMAHW"}
    for blk in nc.m.functions[0].blocks:
        for i in blk.instructions:
            si = i.sync_info
            if si is None or not si.on_wait or len(si.on_wait) < 2:
                continue
            W = list(si.on_wait)
            if i.opcode == "Drain":
                keep = [w for w in W if w.ant_name in out_q]
                assert len(keep) == 1, (i.name, [w.ant_name for w in W])
                si.on_wait = keep
                continue
            if i.opcode == "DMACopy":
                own_q = {u.ant_name for u in (si.on_update or [])
                         if u.ant_name.startswith("DMAHW")}
                W = [w for w in W if w.ant_name not in own_q]
                if len(W) > 1:
                    p = pol_map.get(i.name)
                    if p is None:
                        # keep the engine (producer) wait; input-DMA RAW is
                        # covered through it
                        W = [w for w in W if _eng(w) != "DMAHW"]
                    else:
                        W = [w for w in W if _eng(w) == keep_map[p]]
                assert len(W) <= 1, (i.name, [w.ant_name for w in W])
                si.on_wait = W
                continue
            own = eng_map[str(i.engine).split(".")[-1]]
            W = [w for w in W if _eng(w) != own]
            if len(W) > 1:
                p = pol_map.get(i.name)
                assert p is not None, (
                    i.name, i.opcode, own, [w.ant_name for w in W])
                if p == "none":
                    W = []
                else:
                    want = keep_map[p]
                    keep = [w for w in W if _eng(w) == want]
                    assert len(keep) == 1, (
                        i.name, p, [w.ant_name for w in W])
                    W = keep
            si.on_wait = W
    for blk in nc.m.functions[0].blocks:
        for i in blk.instructions:
            si = i.sync_info
            assert si is None or not si.on_wait or len(si.on_wait) <= 1, (
                i.name, i.opcode, [w.ant_name for w in si.on_wait])


_PROGRAM = None


def _get_program():
    global _PROGRAM
    if _PROGRAM is None:
        nc, _ = build_v6()
        _PROGRAM = nc
    return _PROGRAM


def make_in_maps(inputs):
    """Host-side prep + shard: returns the per-core input maps."""
    x = np.ascontiguousarray(
        np.asarray(inputs["padded_embeddings"], dtype=np.float32)
    ).astype(np.float16)
    lengths = np.asarray(inputs["lengths"]).astype(np.float32)
    w = np.asarray(inputs["w_att"], dtype=np.float32)
    gamma = np.asarray(inputs["ln_gamma"], dtype=np.float32)
    beta = np.asarray(inputs["ln_beta"], dtype=np.float32)
    # b_att shifts every unmasked score equally; softmax cancels it.

    cb16 = np.zeros((128, F16TOT), dtype=np.float16)
    cb16[:, O_W:O_W + D] = w[None, :].astype(np.float16)
    cb16[:, O_GB:O_GB + D] = gamma[None, :].astype(np.float16)
    cb16[:, O_BB:O_BB + D] = beta[None, :].astype(np.float16)

    in_maps = []
    for i in range(N_CORES):
        s = slice(i * B_SHARD, (i + 1) * B_SHARD)
        len_core = lengths[s].reshape(N_BLK, 128).T
        cb32 = np.zeros((128, F32TOT), dtype=np.float32)
        cb32[:, O_IOTA:O_IOTA + L] = np.arange(L, dtype=np.float32)[None, :]
        cb32[:, O_LEN:O_LEN + N_BLK] = len_core
        in_maps.append({"x": x[s], "cb16": cb16, "cb32": cb32})
    return in_maps


def kernel(**inputs):
    from concourse.bass_utils import run_bass_kernel_spmd

    nc = _get_program()
    in_maps = make_in_maps(inputs)
    res = run_bass_kernel_spmd(nc, in_maps, core_ids=list(range(N_CORES)))
    return np.concatenate(
        [res.results[i]["out"] for i in range(N_CORES)], axis=0
    ).astype(np.float32)


def _build_null_program():
    """Same external inputs/outputs, trivial body — for baseline timing."""
    import concourse.bass as bass
    import concourse.tile as tile
    import concourse.mybir as mybir

    f32 = mybir.dt.float32
    f16 = mybir.dt.float16
    nc = bass.Bass("TRN2", target_bir_lowering=False, debug=False)
    nc.dram_tensor("x", [B_SHARD, L, D], f16, kind="ExternalInput")
    nc.dram_tensor("cb16", [128, F16TOT], f16, kind="ExternalInput")
    cb32_d = nc.dram_tensor("cb32", [128, F32TOT], f32, kind="ExternalInput")
    out_d = nc.dram_tensor("out", [B_SHARD, D], f32, kind="ExternalOutput")
    with tile.TileContext(nc) as tc:
        with tc.tile_pool(name="p", bufs=1) as p:
            t = p.tile([128, 128], f32, tag="t")
            nc.sync.dma_start(t[:], cb32_d.ap()[:, 0:128])
            pj = p.tile([128, 1], f32, tag="pj")
            nc.vector.tensor_copy(pj[:], t[:, 0:1])
            o_all = p.tile([128, N_BLK * D], f32, tag="o_all")
            for blk in range(N_BLK):
                nc.vector.tensor_copy(o_all[:, blk * D:(blk + 1) * D], t[:])
            out_dma = nc.sync.dma_start(
                out_d.ap().rearrange("(blk p) d -> p blk d", p=128), o_all[:]
            )
    fix_waits(nc, out_dma, [])
    return nc


def _timed_spmd(nc, in_maps, iters):
    """Repeat execution with device-resident inputs; returns per-iter ns."""
    import time
    import jax
    from jax.sharding import Mesh, NamedSharding, PartitionSpec
    from jax.experimental.shard_map import shard_map
    from concourse import bass2jax
    import concourse.mybir as mybir

    bass2jax.install_neuronx_cc_hook()
    partition_name = nc.partition_id_tensor.name if nc.partition_id_tensor else None
    in_names, out_names, out_avals, zero_outs = [], [], [], []
    for alloc in nc.m.functions[0].allocations:
        if not isinstance(alloc, mybir.MemoryLocationSet):
            continue
        name = alloc.memorylocations[0].name
        if alloc.kind == "ExternalInput":
            if name != partition_name:
                in_names.append(name)
        elif alloc.kind == "ExternalOutput":
            out_names.append(name)
            shape = tuple(alloc.tensor_shape)
            dtype = mybir.dt.np(alloc.dtype)
            out_avals.append(jax.core.ShapedArray(shape, dtype))
            zero_outs.append(np.zeros(shape, dtype))
    n_params = len(in_names)
    n_outs = len(out_avals)
    all_names = list(in_names) + list(out_names)
    if partition_name is not None:
        all_names.append(partition_name)

    def _body(*args):
        operands = list(args)
        if partition_name is not None:
            operands.append(bass2jax.partition_id_tensor())
        return tuple(bass2jax._bass_exec_p.bind(
            *operands,
            out_avals=tuple(out_avals),
            in_names=tuple(all_names),
            out_names=tuple(out_names),
            lowering_input_output_aliases=(),
            sim_require_finite=True,
            sim_require_nnan=True,
            nc=nc,
        ))

    n_cores = len(in_maps)
    devices = jax.devices()[:n_cores]
    mesh = Mesh(np.asarray(devices), ("core",))
    in_specs = (PartitionSpec("core"),) * (n_params + n_outs)
    out_specs = (PartitionSpec("core"),) * n_outs
    donate = tuple(range(n_params, n_params + n_outs))
    sharded = jax.jit(
        shard_map(_body, mesh=mesh, in_specs=in_specs, out_specs=out_specs,
                  check_rep=False),
        donate_argnums=donate,
        keep_unused=True,
    )
    shd = NamedSharding(mesh, PartitionSpec("core"))
    concat_in = [
        jax.device_put(
            np.concatenate(
                [np.asarray(in_maps[c][nm]) for c in range(n_cores)], axis=0
            ),
            shd,
        )
        for nm in in_names
    ]
    times = []
    outs = None
    for _ in range(iters):
        concat_zeros = [
            jax.device_put(
                np.zeros((n_cores * z.shape[0], *z.shape[1:]), z.dtype), shd
            )
            for z in zero_outs
        ]
        jax.block_until_ready(concat_zeros)
        t0 = time.perf_counter()
        outs = sharded(*concat_in, *concat_zeros)
        jax.block_until_ready(outs)
        times.append((time.perf_counter() - t0) * 1e9)
    return times, outs, out_names, out_avals


def bench(inputs, iters=8):
    """Returns (est_kernel_ns, raw_times, null_times, output_array)."""
    nc = _get_program()
    in_maps = make_in_maps(inputs)
    times, outs, out_names, out_avals = _timed_spmd(nc, in_maps, iters)

    null_nc = _build_null_program()
    null_times, _, _, _ = _timed_spmd(null_nc, in_maps, iters)

    est = max(0.0, min(times) - min(null_times))
    out = np.asarray(outs[0]).reshape(N_CORES, *out_avals[0].shape)
    out = np.concatenate([out[i] for i in range(N_CORES)], axis=0)
    return est, times, null_times, out

